# revision 1
# baseline (speedup 1.0000x reference)
"""Multi-head attention (B=2, L=4096, C=512, H=8, Dh=64) on 8 trn2 cores.

Sharding: data-parallel over batch (4 cores per batch element) x
tensor-parallel over heads (2 heads per core). Each core computes a partial
output projection; the host sums the 4 partials per batch element and adds
the bias.

Per-core kernel (scores never hit HBM):
  - inputs: xT [512, 4096] (= x[b].T), wq/wk/wv [512, 128] col slices
    (1/sqrt(Dh) folded into wq), wo [128, 512] row slice
  - Q^T, K^T [128, 4096] bf16 via lhsT=W-chunk (f32r), rhs=xT (f32r)
  - V [4096, 128] bf16 stored per head as [V_h(64) | ones(1) | pad(63)]
    so the AV matmul has 128 weight columns (FWL) and produces the softmax
    denominator in output partition 64
  - per q-chunk of 1024, per k-tile, both heads: S^T[k,q] [128,1024] PSUM
    (2 bf16 matmuls each, K=64, head0/head1 at row groups 0/64 so they run
    concurrently on the PE), one ACT exp each -> P^T bf16, then 2 AV bf16
    matmuls each accumulating att^T [128,1024] PSUM over the 32 k-tiles
  - normalize via reciprocal + partition_broadcast + DVE mult -> attn f32r
  - out-proj: out[q,:] = attn.T @ wo, one f32r matmul per 128-row q-tile
"""

import ml_dtypes
import numpy as np

B, L, C, H = 2, 4096, 512, 8
DH = C // H  # 64
P = 128
NCORES = 8
HEADS_PER_CORE = 2
CORES_PER_BATCH = 4

QCHUNK = 1024  # q columns per attention block (2 PSUM banks)
NQC = L // QCHUNK  # 4
NKT = L // P  # 32 k-tiles
NCC = C // P  # 4 contraction chunks for projections

_cached = {}


def _build(reps=1):
    import concourse.mybir as mybir
    import concourse.tile as tile
    from concourse import bacc

    F32R = mybir.dt.float32r
    F32 = mybir.dt.float32
    BF16 = mybir.dt.bfloat16
    EXP = mybir.ActivationFunctionType.Exp
    MULT = mybir.AluOpType.mult

    nc = bacc.Bacc("TRN2", target_bir_lowering=False, debug=False,
                   num_devices=NCORES)
    xT = nc.dram_tensor("xT", [C, L], BF16, kind="ExternalInput").ap()
    wq = nc.dram_tensor("wq", [C, P], BF16, kind="ExternalInput").ap()
    wk = nc.dram_tensor("wk", [C, P], BF16, kind="ExternalInput").ap()
    wv = nc.dram_tensor("wv", [C, P], BF16, kind="ExternalInput").ap()
    wo = nc.dram_tensor("wo", [P, C], BF16, kind="ExternalInput").ap()
    out = nc.dram_tensor("out", [L, C], BF16, kind="ExternalOutput").ap()

    with tile.TileContext(nc) as tc:
        import contextlib
        loop_cm = tc.For_i(0, reps, 1) if reps > 1 else contextlib.nullcontext()
        with (
            tc.tile_pool(name="persist", bufs=1) as persist,
            tc.tile_pool(name="xpool", bufs=1) as xpool,
            tc.tile_pool(name="ptp", bufs=6) as ptp,
            tc.tile_pool(name="small", bufs=2) as small,
            tc.tile_pool(name="outp", bufs=3) as outp,
            loop_cm,
        ):
            # ---- load inputs ----
            wq_t = persist.tile([P, NCC, P], BF16)
            wk_t = persist.tile([P, NCC, P], BF16)
            wv_t = persist.tile([P, NCC, P], BF16)
            wo_t = persist.tile([P, C], BF16)
            nc.sync.dma_start(wq_t, wq.rearrange("(k p) m -> p k m", p=P))
            nc.sync.dma_start(wk_t, wk.rearrange("(k p) m -> p k m", p=P))
            nc.sync.dma_start(wv_t, wv.rearrange("(k p) m -> p k m", p=P))
            nc.sync.dma_start(wo_t, wo)

            xt = xpool.tile([P, NCC, L], BF16)
            xTr = xT.rearrange("(k p) n -> p k n", p=P)
            for j in range(8):  # split the 8.4MB load across DMA queues
                sl = slice(j * (L // 8), (j + 1) * (L // 8))
                nc.sync.dma_start(xt[:, :, sl], xTr[:, :, sl])

            qT = persist.tile([P, L], BF16)
            kT = persist.tile([P, L], BF16)
            # per head block: [V_h (64) | ones (1) | zero pad (63)]
            v_store = persist.tile([P, NKT, HEADS_PER_CORE, P], BF16)
            attn = persist.tile([P, L], BF16)

            # ---- projections ----
            with tc.tile_pool(name="pj_ps", bufs=2, space="PSUM") as pj_ps:
                # Q^T / K^T: [128 (2 heads x 64), L]
                for dst, w_t in ((qT, wq_t), (kT, wk_t)):
                    for j in range(L // 512):
                        ps = pj_ps.tile([P, 512], F32, tag="qk_ps")
                        for c in range(NCC):
                            nc.tensor.matmul(
                                ps, w_t[:, c, :],
                                xt[:, c, j * 512:(j + 1) * 512],
                                start=(c == 0), stop=(c == NCC - 1),
                            )
                        nc.vector.tensor_copy(
                            dst[:, j * 512:(j + 1) * 512], ps)

                # V: per 128-token tile, [tokens, 128] = xT-chunk.T @ wv
                nc.vector.memset(v_store, 0.0)
                ones_t = small.tile([P, NKT], F32, tag="ones")
                nc.vector.memset(ones_t, 1.0)
                for h in range(HEADS_PER_CORE):
                    nc.vector.tensor_copy(v_store[:, :, h, DH], ones_t)
                for r in range(NKT):
                    ps = pj_ps.tile([P, P], F32, tag="v_ps")
                    for c in range(NCC):
                        nc.tensor.matmul(
                            ps, xt[:, c, r * P:(r + 1) * P], wv_t[:, c, :],
                            start=(c == 0), stop=(c == NCC - 1),
                        )
                    for h in range(HEADS_PER_CORE):
                        nc.vector.tensor_copy(
                            v_store[:, r, h, 0:DH],
                            ps[:, h * DH:(h + 1) * DH])

            # ---- attention ----
            s_ps_cm = tc.tile_pool(name="s_ps", bufs=2, space="PSUM")
            a_ps_cm = tc.tile_pool(name="a_ps", bufs=2, space="PSUM")
            s_ps = s_ps_cm.__enter__()
            a_ps = a_ps_cm.__enter__()
            for qc in range(NQC):
                qsl = slice(qc * QCHUNK, (qc + 1) * QCHUNK)
                att = [a_ps.tile([P, QCHUNK], F32, tag="att", name=f"att{_h}")
                       for _h in range(HEADS_PER_CORE)]
                for kt in range(NKT):
                    sps = [s_ps.tile([P, QCHUNK], F32, tag="spsum",
                                    name=f"sps{_h}")
                           for _h in range(HEADS_PER_CORE)]
                    # scores: head0 at PE rows 0:64, head1 at 64:128 (overlap)
                    for h in range(HEADS_PER_CORE):
                        hsl = slice(h * DH, (h + 1) * DH)
                        for j in range(QCHUNK // 512):
                            nc.tensor.matmul(
                                sps[h][:, j * 512:(j + 1) * 512],
                                kT[hsl, kt * P:(kt + 1) * P],
                                qT[hsl, qc * QCHUNK + j * 512:
                                   qc * QCHUNK + (j + 1) * 512],
                                start=True, stop=True,
                            )
                    pts = []
                    for h in range(HEADS_PER_CORE):
                        pt = ptp.tile([P, QCHUNK], BF16, tag="pt")
                        # split PSUM drain between ACT (direct, ~1.75us) and
                        # DVE copy + SBUF-source exp (1.36 + 0.87us): route
                        # 1/3 direct to balance ACT and DVE busy time
                        if (2 * kt + h) % 3 == 0:
                            nc.scalar.activation(pt, sps[h], EXP)
                        else:
                            scp = ptp.tile([P, QCHUNK], F32, tag="scp")
                            nc.vector.tensor_copy(scp, sps[h])
                            nc.scalar.activation(pt, scp, EXP)
                        pts.append(pt)
                    for h in range(HEADS_PER_CORE):
                        for j in range(QCHUNK // 512):
                            nc.tensor.matmul(
                                att[h][:, j * 512:(j + 1) * 512],
                                v_store[:, kt, h, :],
                                pts[h][:, j * 512:(j + 1) * 512],
                                start=(kt == 0), stop=(kt == NKT - 1),
                            )
                # normalize: recip of denominator row, broadcast, multiply
                for h in range(HEADS_PER_CORE):
                    hsl = slice(h * DH, (h + 1) * DH)
                    recip = small.tile([1, QCHUNK], F32, tag="recip")
                    nc.vector.reciprocal(recip, att[h][DH:DH + 1, :])
                    rb = small.tile([DH, QCHUNK], F32, tag="rb")
                    nc.gpsimd.partition_broadcast(rb, recip)
                    nc.vector.tensor_tensor(
                        attn[hsl, qsl], att[h][0:DH, :], rb, MULT)
            a_ps_cm.__exit__(None, None, None)
            s_ps_cm.__exit__(None, None, None)

            # ---- output projection ----
            with tc.tile_pool(name="o_ps", bufs=3, space="PSUM") as o_ps:
                for qt in range(L // P):
                    ps = o_ps.tile([P, C], F32, tag="o_ps")
                    nc.tensor.matmul(ps, attn[:, qt * P:(qt + 1) * P], wo_t,
                                     start=True, stop=True)
                    osb = outp.tile([P, C], BF16, tag="osb")
                    nc.vector.tensor_copy(osb, ps)
                    nc.sync.dma_start(out[qt * P:(qt + 1) * P, :], osb)

    nc.compile()
    return nc


def _get_nc(reps=1):
    key = f"nc{reps}"
    if key not in _cached:
        _cached[key] = _build(reps)
    return _cached[key]


def _build_in_maps(inputs):
    x = np.asarray(inputs["x"], dtype=np.float32)
    Wq = np.asarray(inputs["Wq"], dtype=np.float32)
    Wk = np.asarray(inputs["Wk"], dtype=np.float32)
    Wv = np.asarray(inputs["Wv"], dtype=np.float32)
    Wo = np.asarray(inputs["Wo"], dtype=np.float32)

    scale = np.float32(1.0 / np.sqrt(DH))
    in_maps = []
    for core in range(NCORES):
        b = core // CORES_PER_BATCH
        j = core % CORES_PER_BATCH
        csl = slice(j * P, (j + 1) * P)
        bf = ml_dtypes.bfloat16
        in_maps.append({
            "xT": np.ascontiguousarray(x[b].T.astype(bf)),
            "wq": np.ascontiguousarray((Wq[:, csl] * scale).astype(bf)),
            "wk": np.ascontiguousarray(Wk[:, csl].astype(bf)),
            "wv": np.ascontiguousarray(Wv[:, csl].astype(bf)),
            "wo": np.ascontiguousarray(Wo[csl, :].astype(bf)),
        })
    return in_maps


def kernel(x, Wq, Wk, Wv, Wo, bo):
    from concourse import bass_utils

    bo = np.asarray(bo, dtype=np.float32)
    in_maps = _build_in_maps(
        {"x": x, "Wq": Wq, "Wk": Wk, "Wv": Wv, "Wo": Wo})

    res = bass_utils.run_bass_kernel_spmd(
        _get_nc(), in_maps, core_ids=list(range(NCORES)))

    out = np.zeros((B, L, C), dtype=np.float32)
    for core in range(NCORES):
        out[core // CORES_PER_BATCH] += res.results[core]["out"].astype(np.float32)
    out += bo[None, None, :]
    return out



# revision 4
# speedup vs baseline: 1.6120x; 1.6120x over previous
"""Multi-head attention (B=2, L=4096, C=512, H=8, Dh=64) on 8 trn2 cores.

Sharding: data-parallel over batch (4 cores per batch element) x
tensor-parallel over heads (2 heads per core). Each core computes a partial
output projection; the host sums the 4 partials per batch element and adds
the bias.

Per-core kernel (scores never hit HBM):
  - inputs: xT [512, 4096] (= x[b].T), wq/wk/wv [512, 128] col slices
    (1/sqrt(Dh) folded into wq), wo [128, 512] row slice
  - Q^T, K^T [128, 4096] bf16 (2 heads x 64 rows)
  - V stored per (k-tile, head) as [128 tokens, 64 dh | 1 ones] so the AV
    matmul emits the softmax denominator in output partition 64
  - per q-chunk of 512: both heads' score tiles live in ONE [128, 2, 512]
    PSUM tile (2 banks); the two score matmuls use PE row groups 0:64 and
    64:128 so they run concurrently; ONE ScalarE exp [128, 1024] drains
    PSUM directly to bf16 SBUF (no DVE copy at all)
  - AV: per (kt, head) one K=128 matmul accumulating att [65, 2, 512] PSUM
  - normalize: reciprocal_approx_fast on the denominator row + gpsimd
    partition_broadcast + DVE multiply -> attn bf16
  - out-proj: out[q,:] = attn.T @ wo per 128-row q-tile
"""

import ml_dtypes
import numpy as np

B, L, C, H = 2, 4096, 512, 8
DH = C // H  # 64
P = 128
NCORES = 8
HEADS_PER_CORE = 2
CORES_PER_BATCH = 4

QCHUNK = 512  # q columns per attention block (1 PSUM bank per head)
NQC = L // QCHUNK  # 8
NKT = L // P  # 32 k-tiles
NCC = C // P  # 4 contraction chunks for projections
VW = DH + 1  # 65: V columns + ones column (denominator)

_cached = {}


def _build(reps=1):
    import concourse.mybir as mybir
    import concourse.tile as tile
    from concourse import bacc

    F32 = mybir.dt.float32
    BF16 = mybir.dt.bfloat16
    EXP = mybir.ActivationFunctionType.Exp
    MULT = mybir.AluOpType.mult

    nc = bacc.Bacc("TRN2", target_bir_lowering=False, debug=False,
                   num_devices=NCORES)
    xT = nc.dram_tensor("xT", [C, L], BF16, kind="ExternalInput").ap()
    wq = nc.dram_tensor("wq", [C, P], BF16, kind="ExternalInput").ap()
    wk = nc.dram_tensor("wk", [C, P], BF16, kind="ExternalInput").ap()
    wv = nc.dram_tensor("wv", [C, P], BF16, kind="ExternalInput").ap()
    wo = nc.dram_tensor("wo", [P, C], BF16, kind="ExternalInput").ap()
    out = nc.dram_tensor("out", [L, C], BF16, kind="ExternalOutput").ap()

    with tile.TileContext(nc) as tc:
        import contextlib
        loop_cm = tc.For_i(0, reps, 1) if reps > 1 else contextlib.nullcontext()
        with (
            tc.tile_pool(name="persist", bufs=1) as persist,
            tc.tile_pool(name="xpool", bufs=1) as xpool,
            tc.tile_pool(name="ptp", bufs=6) as ptp,
            tc.tile_pool(name="small", bufs=2) as small,
            tc.tile_pool(name="outp", bufs=3) as outp,
            loop_cm,
        ):
            # ---- load inputs ----
            wq_t = persist.tile([P, NCC, P], BF16)
            wk_t = persist.tile([P, NCC, P], BF16)
            wv_t = persist.tile([P, NCC, P], BF16)
            wo_t = persist.tile([P, C], BF16)
            nc.sync.dma_start(wq_t, wq.rearrange("(k p) m -> p k m", p=P))
            nc.sync.dma_start(wk_t, wk.rearrange("(k p) m -> p k m", p=P))
            nc.sync.dma_start(wv_t, wv.rearrange("(k p) m -> p k m", p=P))
            nc.sync.dma_start(wo_t, wo)

            xt = xpool.tile([P, NCC, L], BF16)
            xTr = xT.rearrange("(k p) n -> p k n", p=P)
            for j in range(8):  # split the 8.4MB load across DMA queues
                sl = slice(j * (L // 8), (j + 1) * (L // 8))
                nc.sync.dma_start(xt[:, :, sl], xTr[:, :, sl])

            qT = persist.tile([P, L], BF16)
            kT = persist.tile([P, L], BF16)
            # per (k-tile, head): [V_h (64) | ones (1)]
            v_store = persist.tile([P, NKT, HEADS_PER_CORE, VW], BF16)
            attn = persist.tile([P, L], BF16)

            # ---- projections ----
            with tc.tile_pool(name="pj_ps", bufs=2, space="PSUM") as pj_ps:
                # Q^T / K^T: [128 (2 heads x 64), L]
                for dst, w_t in ((qT, wq_t), (kT, wk_t)):
                    for j in range(L // 512):
                        ps = pj_ps.tile([P, 512], F32, tag="qk_ps")
                        for c in range(NCC):
                            nc.tensor.matmul(
                                ps, w_t[:, c, :],
                                xt[:, c, j * 512:(j + 1) * 512],
                                start=(c == 0), stop=(c == NCC - 1),
                            )
                        nc.vector.tensor_copy(
                            dst[:, j * 512:(j + 1) * 512], ps)

                # V: per 128-token tile, [tokens, 128] = xT-chunk.T @ wv
                ones_t = small.tile([P, NKT], F32, tag="ones")
                nc.vector.memset(ones_t, 1.0)
                for h in range(HEADS_PER_CORE):
                    nc.vector.tensor_copy(v_store[:, :, h, DH], ones_t)
                for r in range(NKT):
                    ps = pj_ps.tile([P, P], F32, tag="v_ps")
                    for c in range(NCC):
                        nc.tensor.matmul(
                            ps, xt[:, c, r * P:(r + 1) * P], wv_t[:, c, :],
                            start=(c == 0), stop=(c == NCC - 1),
                        )
                    for h in range(HEADS_PER_CORE):
                        nc.vector.tensor_copy(
                            v_store[:, r, h, 0:DH],
                            ps[:, h * DH:(h + 1) * DH])

            # ---- attention + out-projection ----
            with (
                tc.tile_pool(name="s_ps", bufs=2, space="PSUM") as s_ps,
                tc.tile_pool(name="a_ps", bufs=1, space="PSUM") as a_ps,
                tc.tile_pool(name="o_ps", bufs=2, space="PSUM") as o_ps,
            ):
                for qc in range(NQC):
                    qsl = slice(qc * QCHUNK, (qc + 1) * QCHUNK)
                    # att rows 0:64 = sum_k P*V_h, row 64 = denominator
                    att = a_ps.tile([P, HEADS_PER_CORE, QCHUNK], F32,
                                    tag="att")
                    for kt in range(NKT):
                        sps = s_ps.tile([P, HEADS_PER_CORE, QCHUNK], F32,
                                        tag="sps")
                        # scores: head0 on PE rows 0:64, head1 on 64:128
                        # (concurrent row groups); different PSUM banks
                        for h in range(HEADS_PER_CORE):
                            hsl = slice(h * DH, (h + 1) * DH)
                            nc.tensor.matmul(
                                sps[:, h, :],
                                kT[hsl, kt * P:(kt + 1) * P],
                                qT[hsl, qsl],
                                start=True, stop=True,
                            )
                        # one exp over both heads' banks, PSUM -> SBUF bf16
                        pt = ptp.tile([P, HEADS_PER_CORE, QCHUNK], BF16,
                                      tag="pt")
                        nc.scalar.activation(pt, sps, EXP)
                        for h in range(HEADS_PER_CORE):
                            nc.tensor.matmul(
                                att[0:VW, h, :],
                                v_store[:, kt, h, :],
                                pt[:, h, :],
                                start=(kt == 0), stop=(kt == NKT - 1),
                            )
                    # normalize: approx-reciprocal of denominator row,
                    # broadcast across partitions, multiply
                    for h in range(HEADS_PER_CORE):
                        hsl = slice(h * DH, (h + 1) * DH)
                        den = small.tile([1, QCHUNK], F32, tag="den")
                        nc.vector.tensor_copy(den, att[DH:DH + 1, h, :])
                        recip = small.tile([1, QCHUNK], F32, tag="recip")
                        nc.vector.reciprocal_approx_fast(recip, den)
                        rb = small.tile([DH, QCHUNK], F32, tag="rb")
                        nc.gpsimd.partition_broadcast(rb, recip)
                        nc.vector.tensor_tensor(
                            attn[hsl, qsl], att[0:DH, h, :], rb, MULT)
                    # out-projection for this q-chunk
                    for qt in range(QCHUNK // P):
                        q0 = qc * QCHUNK + qt * P
                        ps = o_ps.tile([P, C], F32, tag="o_ps")
                        nc.tensor.matmul(ps, attn[:, q0:q0 + P], wo_t,
                                         start=True, stop=True)
                        osb = outp.tile([P, C], BF16, tag="osb")
                        nc.vector.tensor_copy(osb, ps)
                        nc.sync.dma_start(out[q0:q0 + P, :], osb)

    nc.compile()
    return nc


def _get_nc(reps=1):
    key = f"nc{reps}"
    if key not in _cached:
        _cached[key] = _build(reps)
    return _cached[key]


def _build_in_maps(inputs):
    x = np.asarray(inputs["x"], dtype=np.float32)
    Wq = np.asarray(inputs["Wq"], dtype=np.float32)
    Wk = np.asarray(inputs["Wk"], dtype=np.float32)
    Wv = np.asarray(inputs["Wv"], dtype=np.float32)
    Wo = np.asarray(inputs["Wo"], dtype=np.float32)

    scale = np.float32(1.0 / np.sqrt(DH))
    in_maps = []
    for core in range(NCORES):
        b = core // CORES_PER_BATCH
        j = core % CORES_PER_BATCH
        csl = slice(j * P, (j + 1) * P)
        bf = ml_dtypes.bfloat16
        in_maps.append({
            "xT": np.ascontiguousarray(x[b].T.astype(bf)),
            "wq": np.ascontiguousarray((Wq[:, csl] * scale).astype(bf)),
            "wk": np.ascontiguousarray(Wk[:, csl].astype(bf)),
            "wv": np.ascontiguousarray(Wv[:, csl].astype(bf)),
            "wo": np.ascontiguousarray(Wo[csl, :].astype(bf)),
        })
    return in_maps


def kernel(x, Wq, Wk, Wv, Wo, bo):
    from concourse import bass_utils

    bo = np.asarray(bo, dtype=np.float32)
    in_maps = _build_in_maps(
        {"x": x, "Wq": Wq, "Wk": Wk, "Wv": Wv, "Wo": Wo})

    res = bass_utils.run_bass_kernel_spmd(
        _get_nc(), in_maps, core_ids=list(range(NCORES)))

    out = np.zeros((B, L, C), dtype=np.float32)
    for core in range(NCORES):
        out[core // CORES_PER_BATCH] += res.results[core]["out"].astype(np.float32)
    out += bo[None, None, :]
    return out


# revision 9
# speedup vs baseline: 1.7740x; 1.1005x over previous
"""Multi-head attention (B=2, L=4096, C=512, H=8, Dh=64) on 8 trn2 cores.

Sharding: data-parallel over batch (4 cores per batch element) x
tensor-parallel over heads (2 heads per core). Each core computes a partial
output projection; the host sums the 4 partials per batch element and adds
the bias.

Per-core kernel (scores never hit HBM):
  - inputs: xT [512, 4096] (= x[b].T), wq/wk/wv [512, 128] col slices
    (1/sqrt(Dh) folded into wq), wo [128, 512] row slice
  - Q^T, K^T [128, 4096] bf16 (2 heads x 64 rows)
  - V stored per (k-tile, head) as [128 tokens, 64 dh | 1 ones] so the AV
    matmul emits the softmax denominator in output partition 64
  - per q-chunk of 512: both heads' score tiles live in ONE [128, 2, 512]
    PSUM tile (2 banks); the two score matmuls use PE row groups 0:64 and
    64:128 so they run concurrently; ONE ScalarE exp [128, 1024] drains
    PSUM directly to bf16 SBUF (no DVE copy at all)
  - AV: per (kt, head) one K=128 matmul accumulating att [65, 2, 512] PSUM
  - normalize: reciprocal_approx_fast on the denominator row + gpsimd
    partition_broadcast + DVE multiply -> attn bf16
  - out-proj: out[q,:] = attn.T @ wo per 128-row q-tile
"""

import ml_dtypes
import numpy as np

B, L, C, H = 2, 4096, 512, 8
DH = C // H  # 64
P = 128
NCORES = 8
HEADS_PER_CORE = 2
CORES_PER_BATCH = 4

QCHUNK = 512  # q columns per attention block (1 PSUM bank per head)
NQC = L // QCHUNK  # 8
NKT = L // P  # 32 k-tiles
NCC = C // P  # 4 contraction chunks for projections
VW = DH + 1  # 65: V columns + ones column (denominator)

_cached = {}


def _build(reps=1):
    import concourse.mybir as mybir
    import concourse.tile as tile
    from concourse import bacc

    F32 = mybir.dt.float32
    BF16 = mybir.dt.bfloat16
    EXP = mybir.ActivationFunctionType.Exp
    MULT = mybir.AluOpType.mult

    nc = bacc.Bacc("TRN2", target_bir_lowering=False, debug=False,
                   num_devices=NCORES)
    xT = nc.dram_tensor("xT", [C, L], BF16, kind="ExternalInput").ap()
    wq = nc.dram_tensor("wq", [C, P], BF16, kind="ExternalInput").ap()
    wk = nc.dram_tensor("wk", [C, P], BF16, kind="ExternalInput").ap()
    wv = nc.dram_tensor("wv", [C, P], BF16, kind="ExternalInput").ap()
    wo = nc.dram_tensor("wo", [P, C], BF16, kind="ExternalInput").ap()
    out = nc.dram_tensor("out", [L, C], BF16, kind="ExternalOutput").ap()

    with tile.TileContext(nc) as tc:
        import contextlib
        loop_cm = tc.For_i(0, reps, 1) if reps > 1 else contextlib.nullcontext()
        with (
            tc.tile_pool(name="persist", bufs=1) as persist,
            tc.tile_pool(name="xpool", bufs=1) as xpool,
            tc.tile_pool(name="ptp", bufs=6) as ptp,
            tc.tile_pool(name="small", bufs=2) as small,
            tc.tile_pool(name="outp", bufs=3) as outp,
            loop_cm,
        ):
            # ---- load inputs ----
            wq_t = persist.tile([P, NCC, P], BF16)
            wk_t = persist.tile([P, NCC, P], BF16)
            wv_t = persist.tile([P, NCC, P], BF16)
            wo_t = persist.tile([P, C], BF16)
            nc.sync.dma_start(wq_t, wq.rearrange("(k p) m -> p k m", p=P))
            nc.sync.dma_start(wk_t, wk.rearrange("(k p) m -> p k m", p=P))
            nc.sync.dma_start(wv_t, wv.rearrange("(k p) m -> p k m", p=P))
            nc.sync.dma_start(wo_t, wo)

            xt = xpool.tile([P, NCC, L], BF16)
            xTr = xT.rearrange("(k p) n -> p k n", p=P)
            for j in range(8):  # split the 8.4MB load across DMA queues
                sl = slice(j * (L // 8), (j + 1) * (L // 8))
                nc.sync.dma_start(xt[:, :, sl], xTr[:, :, sl])

            qT = persist.tile([P, L], BF16)
            kT = persist.tile([P, L], BF16)
            # per (k-tile, head): [V_h (64) | ones (1)]
            v_store = persist.tile([P, NKT, HEADS_PER_CORE, VW], BF16)
            attn = persist.tile([P, L], BF16)

            # ---- Q/K projections ----
            with tc.tile_pool(name="pj_ps", bufs=2, space="PSUM") as pj_ps:
                # Q^T / K^T: [128 (2 heads x 64), L]
                for dst, w_t in ((qT, wq_t), (kT, wk_t)):
                    for j in range(L // 512):
                        ps = pj_ps.tile([P, 512], F32, tag="qk_ps")
                        for c in range(NCC):
                            nc.tensor.matmul(
                                ps, w_t[:, c, :],
                                xt[:, c, j * 512:(j + 1) * 512],
                                start=(c == 0), stop=(c == NCC - 1),
                            )
                        nc.vector.tensor_copy(
                            dst[:, j * 512:(j + 1) * 512], ps)

                ones_t = small.tile([P, NKT], F32, tag="ones")
                nc.vector.memset(ones_t, 1.0)
                for h in range(HEADS_PER_CORE):
                    nc.vector.tensor_copy(v_store[:, :, h, DH], ones_t)

            # ---- attention + V-projection (interleaved) + out-proj ----
            with (
                tc.tile_pool(name="s_ps", bufs=2, space="PSUM") as s_ps,
                tc.tile_pool(name="a_ps", bufs=1, space="PSUM") as a_ps,
                tc.tile_pool(name="v_ps", bufs=2, space="PSUM") as v_ps,
            ):
                def v_proj(r):
                    # V tile r: [128 tokens, 128 (2 heads x 64)]
                    ps = v_ps.tile([P, C], F32, tag="vo_ps", name="vps")
                    ps = ps[:, 0:P]
                    for c in range(NCC):
                        nc.tensor.matmul(
                            ps, xt[:, c, r * P:(r + 1) * P], wv_t[:, c, :],
                            start=(c == 0), stop=(c == NCC - 1),
                        )
                    for h in range(HEADS_PER_CORE):
                        nc.vector.tensor_copy(
                            v_store[:, r, h, 0:DH],
                            ps[:, h * DH:(h + 1) * DH])

                def out_proj(qc, qt):
                    q0 = qc * QCHUNK + qt * P
                    ps = v_ps.tile([P, C], F32, tag="vo_ps")
                    nc.tensor.matmul(ps, attn[:, q0:q0 + P], wo_t,
                                     start=True, stop=True)
                    osb = outp.tile([P, C], BF16, tag="osb")
                    nc.vector.tensor_copy(osb, ps)
                    nc.sync.dma_start(out[q0:q0 + P, :], osb)

                for qc in range(NQC):
                    qsl = slice(qc * QCHUNK, (qc + 1) * QCHUNK)
                    # att rows 0:64 = sum_k P*V_h, row 64 = denominator
                    att = a_ps.tile([P, HEADS_PER_CORE, QCHUNK], F32,
                                    tag="att")
                    for kt in range(NKT):
                        if qc == 0:
                            v_proj(kt)  # overlap V-proj with first q-chunk
                        elif kt < QCHUNK // P:
                            out_proj(qc - 1, kt)  # deferred out-projection
                        sps = s_ps.tile([P, HEADS_PER_CORE, QCHUNK], F32,
                                        tag="sps")
                        # scores: head0 on PE rows 0:64, head1 on 64:128
                        # (concurrent row groups); different PSUM banks
                        for h in range(HEADS_PER_CORE):
                            hsl = slice(h * DH, (h + 1) * DH)
                            nc.tensor.matmul(
                                sps[:, h, :],
                                kT[hsl, kt * P:(kt + 1) * P],
                                qT[hsl, qsl],
                                start=True, stop=True,
                            )
                        # one exp over both heads' banks, PSUM -> SBUF bf16
                        pt = ptp.tile([P, HEADS_PER_CORE, QCHUNK], BF16,
                                      tag="pt")
                        nc.scalar.activation(pt, sps, EXP)
                        for h in range(HEADS_PER_CORE):
                            nc.tensor.matmul(
                                att[0:VW, h, :],
                                v_store[:, kt, h, :],
                                pt[:, h, :],
                                start=(kt == 0), stop=(kt == NKT - 1),
                            )
                    # stage att to SBUF with one copy so the PSUM slot
                    # frees immediately; normalize from SBUF off-path
                    attsb = small.tile([VW, HEADS_PER_CORE, QCHUNK], F32,
                                       tag="attsb")
                    nc.vector.tensor_copy(attsb, att[0:VW])
                    # both heads' denominator rows -> partition 0, one
                    # approx-reciprocal (reciprocal_approx_fast requires
                    # base_partition 0)
                    den2 = small.tile([1, HEADS_PER_CORE, QCHUNK], F32,
                                      tag="den2")
                    nc.vector.tensor_copy(den2, attsb[DH:DH + 1, :, :])
                    recip2 = small.tile([1, HEADS_PER_CORE, QCHUNK], F32,
                                        tag="recip2")
                    nc.vector.reciprocal_approx_fast(recip2, den2)
                    for h in range(HEADS_PER_CORE):
                        hsl = slice(h * DH, (h + 1) * DH)
                        rb = small.tile([DH, QCHUNK], F32, tag="rb")
                        nc.gpsimd.partition_broadcast(rb, recip2[:, h, :])
                        nc.vector.tensor_tensor(
                            attn[hsl, qsl], attsb[0:DH, h, :], rb, MULT)
                for qt in range(QCHUNK // P):
                    out_proj(NQC - 1, qt)

    nc.compile()
    return nc


def _get_nc(reps=1):
    key = f"nc{reps}"
    if key not in _cached:
        _cached[key] = _build(reps)
    return _cached[key]


def _build_in_maps(inputs):
    x = np.asarray(inputs["x"], dtype=np.float32)
    Wq = np.asarray(inputs["Wq"], dtype=np.float32)
    Wk = np.asarray(inputs["Wk"], dtype=np.float32)
    Wv = np.asarray(inputs["Wv"], dtype=np.float32)
    Wo = np.asarray(inputs["Wo"], dtype=np.float32)

    scale = np.float32(1.0 / np.sqrt(DH))
    in_maps = []
    for core in range(NCORES):
        b = core // CORES_PER_BATCH
        j = core % CORES_PER_BATCH
        csl = slice(j * P, (j + 1) * P)
        bf = ml_dtypes.bfloat16
        in_maps.append({
            "xT": np.ascontiguousarray(x[b].T.astype(bf)),
            "wq": np.ascontiguousarray((Wq[:, csl] * scale).astype(bf)),
            "wk": np.ascontiguousarray(Wk[:, csl].astype(bf)),
            "wv": np.ascontiguousarray(Wv[:, csl].astype(bf)),
            "wo": np.ascontiguousarray(Wo[csl, :].astype(bf)),
        })
    return in_maps


def kernel(x, Wq, Wk, Wv, Wo, bo):
    from concourse import bass_utils

    bo = np.asarray(bo, dtype=np.float32)
    in_maps = _build_in_maps(
        {"x": x, "Wq": Wq, "Wk": Wk, "Wv": Wv, "Wo": Wo})

    res = bass_utils.run_bass_kernel_spmd(
        _get_nc(), in_maps, core_ids=list(range(NCORES)))

    out = np.zeros((B, L, C), dtype=np.float32)
    for core in range(NCORES):
        out[core // CORES_PER_BATCH] += res.results[core]["out"].astype(np.float32)
    out += bo[None, None, :]
    return out


# revision 11
# speedup vs baseline: 1.7960x; 1.0124x over previous
"""Multi-head attention (B=2, L=4096, C=512, H=8, Dh=64) on 8 trn2 cores.

Sharding: data-parallel over batch (4 cores per batch element) x
tensor-parallel over heads (2 heads per core). Each core computes a partial
output projection; the host sums the 4 partials per batch element and adds
the bias.

Per-core kernel (scores never hit HBM):
  - inputs: xT [512, 4096] (= x[b].T), wq/wk/wv [512, 128] col slices
    (1/sqrt(Dh) folded into wq), wo [128, 512] row slice
  - Q^T, K^T [128, 4096] bf16 (2 heads x 64 rows)
  - V stored per (k-tile, head) as [128 tokens, 64 dh | 1 ones] so the AV
    matmul emits the softmax denominator in output partition 64
  - per q-chunk of 512: both heads' score tiles live in ONE [128, 2, 512]
    PSUM tile (2 banks); the two score matmuls use PE row groups 0:64 and
    64:128 so they run concurrently; ONE ScalarE exp [128, 1024] drains
    PSUM directly to bf16 SBUF (no DVE copy at all)
  - AV: per (kt, head) one K=128 matmul accumulating att [65, 2, 512] PSUM
  - normalize: reciprocal_approx_fast on the denominator row + gpsimd
    partition_broadcast + DVE multiply -> attn bf16
  - out-proj: out[q,:] = attn.T @ wo per 128-row q-tile
"""

import ml_dtypes
import numpy as np

B, L, C, H = 2, 4096, 512, 8
DH = C // H  # 64
P = 128
NCORES = 8
HEADS_PER_CORE = 2
CORES_PER_BATCH = 4

QCHUNK = 512  # q columns per attention block (1 PSUM bank per head)
NQC = L // QCHUNK  # 8
NKT = L // P  # 32 k-tiles
NCC = C // P  # 4 contraction chunks for projections
VW = DH + 1  # 65: V columns + ones column (denominator)

_cached = {}


def _build(reps=1):
    import concourse.mybir as mybir
    import concourse.tile as tile
    from concourse import bacc

    F32 = mybir.dt.float32
    BF16 = mybir.dt.bfloat16
    U16 = mybir.dt.uint16
    EXP = mybir.ActivationFunctionType.Exp
    MULT = mybir.AluOpType.mult
    ADD = mybir.AluOpType.add
    # Schraudolph bf16 exp: bitcast(u16(round(s*A + B))) ~= exp(s)
    SCH_A = 128.0 / float(np.log(2.0))
    SCH_B = 127.0 * 128.0 - 5.59

    nc = bacc.Bacc("TRN2", target_bir_lowering=False, debug=False,
                   num_devices=NCORES)
    xT = nc.dram_tensor("xT", [C, L], BF16, kind="ExternalInput").ap()
    wq = nc.dram_tensor("wq", [C, P], BF16, kind="ExternalInput").ap()
    wk = nc.dram_tensor("wk", [C, P], BF16, kind="ExternalInput").ap()
    wv = nc.dram_tensor("wv", [C, P], BF16, kind="ExternalInput").ap()
    wo = nc.dram_tensor("wo", [P, C], BF16, kind="ExternalInput").ap()
    out = nc.dram_tensor("out", [L, C], BF16, kind="ExternalOutput").ap()

    with tile.TileContext(nc) as tc:
        import contextlib
        loop_cm = tc.For_i(0, reps, 1) if reps > 1 else contextlib.nullcontext()
        with (
            tc.tile_pool(name="persist", bufs=1) as persist,
            tc.tile_pool(name="xpool", bufs=1) as xpool,
            tc.tile_pool(name="ptp", bufs=6) as ptp,
            tc.tile_pool(name="small", bufs=2) as small,
            tc.tile_pool(name="outp", bufs=3) as outp,
            loop_cm,
        ):
            # ---- load inputs ----
            wq_t = persist.tile([P, NCC, P], BF16)
            wk_t = persist.tile([P, NCC, P], BF16)
            wv_t = persist.tile([P, NCC, P], BF16)
            wo_t = persist.tile([P, C], BF16)
            nc.sync.dma_start(wq_t, wq.rearrange("(k p) m -> p k m", p=P))
            nc.sync.dma_start(wk_t, wk.rearrange("(k p) m -> p k m", p=P))
            nc.sync.dma_start(wv_t, wv.rearrange("(k p) m -> p k m", p=P))
            nc.sync.dma_start(wo_t, wo)

            xt = xpool.tile([P, NCC, L], BF16)
            xTr = xT.rearrange("(k p) n -> p k n", p=P)
            for j in range(8):  # split the 8.4MB load across DMA queues
                sl = slice(j * (L // 8), (j + 1) * (L // 8))
                nc.sync.dma_start(xt[:, :, sl], xTr[:, :, sl])

            qT = persist.tile([P, L], BF16)
            kT = persist.tile([P, L], BF16)
            # per (k-tile, head): [V_h (64) | ones (1)]
            v_store = persist.tile([P, NKT, HEADS_PER_CORE, VW], BF16)
            attn = persist.tile([P, L], BF16)

            # ---- Q/K projections ----
            with tc.tile_pool(name="pj_ps", bufs=2, space="PSUM") as pj_ps:
                # Q^T / K^T: [128 (2 heads x 64), L]
                for dst, w_t in ((qT, wq_t), (kT, wk_t)):
                    for j in range(L // 512):
                        ps = pj_ps.tile([P, 512], F32, tag="qk_ps")
                        for c in range(NCC):
                            nc.tensor.matmul(
                                ps, w_t[:, c, :],
                                xt[:, c, j * 512:(j + 1) * 512],
                                start=(c == 0), stop=(c == NCC - 1),
                            )
                        nc.vector.tensor_copy(
                            dst[:, j * 512:(j + 1) * 512], ps)

                ones_t = small.tile([P, NKT], F32, tag="ones")
                nc.vector.memset(ones_t, 1.0)
                for h in range(HEADS_PER_CORE):
                    nc.vector.tensor_copy(v_store[:, :, h, DH], ones_t)

            # ---- attention + V-projection (interleaved) + out-proj ----
            with (
                tc.tile_pool(name="s_ps", bufs=2, space="PSUM") as s_ps,
                tc.tile_pool(name="a_ps", bufs=1, space="PSUM") as a_ps,
                tc.tile_pool(name="v_ps", bufs=2, space="PSUM") as v_ps,
            ):
                def v_proj(r):
                    # V tile r: [128 tokens, 128 (2 heads x 64)]
                    ps = v_ps.tile([P, C], F32, tag="vo_ps", name="vps")
                    ps = ps[:, 0:P]
                    for c in range(NCC):
                        nc.tensor.matmul(
                            ps, xt[:, c, r * P:(r + 1) * P], wv_t[:, c, :],
                            start=(c == 0), stop=(c == NCC - 1),
                        )
                    for h in range(HEADS_PER_CORE):
                        nc.vector.tensor_copy(
                            v_store[:, r, h, 0:DH],
                            ps[:, h * DH:(h + 1) * DH])

                def out_proj(qc, qt):
                    q0 = qc * QCHUNK + qt * P
                    ps = v_ps.tile([P, C], F32, tag="vo_ps")
                    nc.tensor.matmul(ps, attn[:, q0:q0 + P], wo_t,
                                     start=True, stop=True)
                    osb = outp.tile([P, C], BF16, tag="osb")
                    nc.vector.tensor_copy(osb, ps)
                    nc.sync.dma_start(out[q0:q0 + P, :], osb)

                for qc in range(NQC):
                    qsl = slice(qc * QCHUNK, (qc + 1) * QCHUNK)
                    # att rows 0:64 = sum_k P*V_h, row 64 = denominator
                    att = a_ps.tile([P, HEADS_PER_CORE, QCHUNK], F32,
                                    tag="att")
                    pending = []

                    def av(kt, pt_ap):
                        for h in range(HEADS_PER_CORE):
                            nc.tensor.matmul(
                                att[0:VW, h, :],
                                v_store[:, kt, h, :],
                                pt_ap[:, h, :],
                                start=(kt == 0), stop=(kt == NKT - 1),
                            )

                    for kt in range(NKT):
                        if qc == 0:
                            v_proj(kt)  # overlap V-proj with first q-chunk
                        elif kt < QCHUNK // P:
                            out_proj(qc - 1, kt)  # deferred out-projection
                        sps = s_ps.tile([P, HEADS_PER_CORE, QCHUNK], F32,
                                        tag="sps")
                        # scores: head0 on PE rows 0:64, head1 on 64:128
                        # (concurrent row groups); different PSUM banks
                        for h in range(HEADS_PER_CORE):
                            hsl = slice(h * DH, (h + 1) * DH)
                            nc.tensor.matmul(
                                sps[:, h, :],
                                kT[hsl, kt * P:(kt + 1) * P],
                                qT[hsl, qsl],
                                start=True, stop=True,
                            )
                        if kt % 4 == 3:
                            # Schraudolph exp on DVE (offloads ScalarE)
                            ptu = ptp.tile([P, HEADS_PER_CORE, QCHUNK],
                                           U16, tag="ptu")
                            nc.vector.tensor_scalar(
                                ptu, sps, SCH_A, SCH_B, MULT, ADD)
                            pt_ap = ptu[:, :, :].bitcast(BF16)
                        else:
                            # exact exp on ScalarE, PSUM -> SBUF bf16
                            pt = ptp.tile([P, HEADS_PER_CORE, QCHUNK],
                                          BF16, tag="pt")
                            nc.scalar.activation(pt, sps, EXP)
                            pt_ap = pt
                        # AV trails by one k-tile so the PE FIFO never
                        # head-blocks on the next q-chunk's att slot
                        pending.append((kt, pt_ap))
                        if len(pending) > 1:
                            av(*pending.pop(0))
                    av(*pending.pop(0))
                    # stage att to SBUF with one copy so the PSUM slot
                    # frees immediately; normalize from SBUF off-path
                    attsb = small.tile([VW, HEADS_PER_CORE, QCHUNK], F32,
                                       tag="attsb")
                    nc.vector.tensor_copy(attsb, att[0:VW])
                    # both heads' denominator rows -> partition 0, one
                    # approx-reciprocal (reciprocal_approx_fast requires
                    # base_partition 0)
                    den2 = small.tile([1, HEADS_PER_CORE, QCHUNK], F32,
                                      tag="den2")
                    nc.vector.tensor_copy(den2, attsb[DH:DH + 1, :, :])
                    recip2 = small.tile([1, HEADS_PER_CORE, QCHUNK], F32,
                                        tag="recip2")
                    nc.vector.reciprocal_approx_fast(recip2, den2)
                    for h in range(HEADS_PER_CORE):
                        hsl = slice(h * DH, (h + 1) * DH)
                        rb = small.tile([DH, QCHUNK], F32, tag="rb")
                        nc.gpsimd.partition_broadcast(rb, recip2[:, h, :])
                        nc.vector.tensor_tensor(
                            attn[hsl, qsl], attsb[0:DH, h, :], rb, MULT)
                for qt in range(QCHUNK // P):
                    out_proj(NQC - 1, qt)

    nc.compile()
    return nc


def _get_nc(reps=1):
    key = f"nc{reps}"
    if key not in _cached:
        _cached[key] = _build(reps)
    return _cached[key]


def _build_in_maps(inputs):
    x = np.asarray(inputs["x"], dtype=np.float32)
    Wq = np.asarray(inputs["Wq"], dtype=np.float32)
    Wk = np.asarray(inputs["Wk"], dtype=np.float32)
    Wv = np.asarray(inputs["Wv"], dtype=np.float32)
    Wo = np.asarray(inputs["Wo"], dtype=np.float32)

    scale = np.float32(1.0 / np.sqrt(DH))
    in_maps = []
    for core in range(NCORES):
        b = core // CORES_PER_BATCH
        j = core % CORES_PER_BATCH
        csl = slice(j * P, (j + 1) * P)
        bf = ml_dtypes.bfloat16
        in_maps.append({
            "xT": np.ascontiguousarray(x[b].T.astype(bf)),
            "wq": np.ascontiguousarray((Wq[:, csl] * scale).astype(bf)),
            "wk": np.ascontiguousarray(Wk[:, csl].astype(bf)),
            "wv": np.ascontiguousarray(Wv[:, csl].astype(bf)),
            "wo": np.ascontiguousarray(Wo[csl, :].astype(bf)),
        })
    return in_maps


def kernel(x, Wq, Wk, Wv, Wo, bo):
    from concourse import bass_utils

    bo = np.asarray(bo, dtype=np.float32)
    in_maps = _build_in_maps(
        {"x": x, "Wq": Wq, "Wk": Wk, "Wv": Wv, "Wo": Wo})

    res = bass_utils.run_bass_kernel_spmd(
        _get_nc(), in_maps, core_ids=list(range(NCORES)))

    out = np.zeros((B, L, C), dtype=np.float32)
    for core in range(NCORES):
        out[core // CORES_PER_BATCH] += res.results[core]["out"].astype(np.float32)
    out += bo[None, None, :]
    return out


# revision 12
# speedup vs baseline: 1.8647x; 1.0382x over previous
"""Multi-head attention (B=2, L=4096, C=512, H=8, Dh=64) on 8 trn2 cores.

Sharding: data-parallel over batch (4 cores per batch element) x
tensor-parallel over heads (2 heads per core). Each core computes a partial
output projection; the host sums the 4 partials per batch element and adds
the bias.

Per-core kernel (scores never hit HBM):
  - inputs: xT [512, 4096] (= x[b].T), wq/wk/wv [512, 128] col slices
    (1/sqrt(Dh) folded into wq), wo [128, 512] row slice
  - Q^T, K^T [128, 4096] bf16 (2 heads x 64 rows)
  - V stored per (k-tile, head) as [128 tokens, 64 dh | 1 ones] so the AV
    matmul emits the softmax denominator in output partition 64
  - per q-chunk of 512: both heads' score tiles live in ONE [128, 2, 512]
    PSUM tile (2 banks); the two score matmuls use PE row groups 0:64 and
    64:128 so they run concurrently; ONE ScalarE exp [128, 1024] drains
    PSUM directly to bf16 SBUF (no DVE copy at all)
  - AV: per (kt, head) one K=128 matmul accumulating att [65, 2, 512] PSUM
  - normalize: reciprocal_approx_fast on the denominator row + gpsimd
    partition_broadcast + DVE multiply -> attn bf16
  - out-proj: out[q,:] = attn.T @ wo per 128-row q-tile
"""

import ml_dtypes
import numpy as np

B, L, C, H = 2, 4096, 512, 8
DH = C // H  # 64
P = 128
NCORES = 8
HEADS_PER_CORE = 2
CORES_PER_BATCH = 4

QCHUNK = 512  # q columns per attention block (1 PSUM bank per head)
NQC = L // QCHUNK  # 8
NKT = L // P  # 32 k-tiles
NCC = C // P  # 4 contraction chunks for projections
VW = DH + 1  # 65: V columns + ones column (denominator)

_cached = {}


def _build(reps=1):
    import concourse.mybir as mybir
    import concourse.tile as tile
    from concourse import bacc

    F32 = mybir.dt.float32
    BF16 = mybir.dt.bfloat16
    U16 = mybir.dt.uint16
    EXP = mybir.ActivationFunctionType.Exp
    MULT = mybir.AluOpType.mult
    ADD = mybir.AluOpType.add
    # Schraudolph bf16 exp: bitcast(u16(round(s*A + B))) ~= exp(s)
    SCH_A = 128.0 / float(np.log(2.0))
    SCH_B = 127.0 * 128.0 - 5.59

    nc = bacc.Bacc("TRN2", target_bir_lowering=False, debug=False,
                   num_devices=NCORES)
    xT = nc.dram_tensor("xT", [C, L], BF16, kind="ExternalInput").ap()
    wq = nc.dram_tensor("wq", [C, P], BF16, kind="ExternalInput").ap()
    wk = nc.dram_tensor("wk", [C, P], BF16, kind="ExternalInput").ap()
    wv = nc.dram_tensor("wv", [C, P], BF16, kind="ExternalInput").ap()
    wo = nc.dram_tensor("wo", [P, C], BF16, kind="ExternalInput").ap()
    out = nc.dram_tensor("out", [L, C], BF16, kind="ExternalOutput").ap()

    with tile.TileContext(nc) as tc:
        import contextlib
        loop_cm = tc.For_i(0, reps, 1) if reps > 1 else contextlib.nullcontext()
        with (
            tc.tile_pool(name="persist", bufs=1) as persist,
            tc.tile_pool(name="xpool", bufs=1) as xpool,
            tc.tile_pool(name="ptp", bufs=6) as ptp,
            tc.tile_pool(name="small", bufs=2) as small,
            tc.tile_pool(name="outp", bufs=3) as outp,
            loop_cm,
        ):
            # ---- load inputs ----
            wq_t = persist.tile([P, NCC, P], BF16)
            wk_t = persist.tile([P, NCC, P], BF16)
            wv_t = persist.tile([P, NCC, P], BF16)
            wo_t = persist.tile([P, C], BF16)
            nc.sync.dma_start(wq_t, wq.rearrange("(k p) m -> p k m", p=P))
            nc.sync.dma_start(wk_t, wk.rearrange("(k p) m -> p k m", p=P))
            nc.sync.dma_start(wv_t, wv.rearrange("(k p) m -> p k m", p=P))
            nc.sync.dma_start(wo_t, wo)

            xt = xpool.tile([P, NCC, L], BF16)
            xTr = xT.rearrange("(k p) n -> p k n", p=P)
            for j in range(8):  # split the 8.4MB load across DMA queues
                sl = slice(j * (L // 8), (j + 1) * (L // 8))
                nc.sync.dma_start(xt[:, :, sl], xTr[:, :, sl])

            qT = persist.tile([P, L], BF16)
            kT = persist.tile([P, L], BF16)
            # per (k-tile, head): [V_h (64) | ones (1)]
            v_store = persist.tile([P, NKT, HEADS_PER_CORE, VW], BF16)
            attn = persist.tile([P, L], BF16)

            ones_t = small.tile([P, NKT], F32, tag="ones")
            nc.vector.memset(ones_t, 1.0)
            for h in range(HEADS_PER_CORE):
                nc.vector.tensor_copy(v_store[:, :, h, DH], ones_t)

            # ---- attention; Q/K/V projections interleaved into the ----
            # ---- loops so ScalarE exp starts almost immediately     ----
            with (
                tc.tile_pool(name="s_ps", bufs=2, space="PSUM") as s_ps,
                tc.tile_pool(name="a_ps", bufs=1, space="PSUM") as a_ps,
                tc.tile_pool(name="v_ps", bufs=2, space="PSUM") as v_ps,
            ):
                def qk_proj(dst, w_t, j):
                    # one 512-token chunk of Q^T or K^T
                    ps = v_ps.tile([P, C], F32, tag="vo_ps", name="qkps")
                    ps = ps[:, 0:512]
                    for c in range(NCC):
                        nc.tensor.matmul(
                            ps, w_t[:, c, :],
                            xt[:, c, j * 512:(j + 1) * 512],
                            start=(c == 0), stop=(c == NCC - 1),
                        )
                    nc.vector.tensor_copy(dst[:, j * 512:(j + 1) * 512], ps)

                def v_proj(r):
                    # V tile r: [128 tokens, 128 (2 heads x 64)]
                    ps = v_ps.tile([P, C], F32, tag="vo_ps", name="vps")
                    ps = ps[:, 0:P]
                    for c in range(NCC):
                        nc.tensor.matmul(
                            ps, xt[:, c, r * P:(r + 1) * P], wv_t[:, c, :],
                            start=(c == 0), stop=(c == NCC - 1),
                        )
                    for h in range(HEADS_PER_CORE):
                        nc.vector.tensor_copy(
                            v_store[:, r, h, 0:DH],
                            ps[:, h * DH:(h + 1) * DH])

                def out_proj(qc, qt):
                    q0 = qc * QCHUNK + qt * P
                    ps = v_ps.tile([P, C], F32, tag="vo_ps")
                    nc.tensor.matmul(ps, attn[:, q0:q0 + P], wo_t,
                                     start=True, stop=True)
                    osb = outp.tile([P, C], BF16, tag="osb")
                    nc.vector.tensor_copy(osb, ps)
                    nc.sync.dma_start(out[q0:q0 + P, :], osb)

                qk_proj(qT, wq_t, 0)
                qk_proj(kT, wk_t, 0)

                for qc in range(NQC):
                    qsl = slice(qc * QCHUNK, (qc + 1) * QCHUNK)
                    if qc + 1 < NQC:
                        qk_proj(qT, wq_t, qc + 1)  # next q-chunk's Q^T
                    # att rows 0:64 = sum_k P*V_h, row 64 = denominator
                    att = a_ps.tile([P, HEADS_PER_CORE, QCHUNK], F32,
                                    tag="att")
                    pending = []

                    def av(kt, pt_ap):
                        for h in range(HEADS_PER_CORE):
                            nc.tensor.matmul(
                                att[0:VW, h, :],
                                v_store[:, kt, h, :],
                                pt_ap[:, h, :],
                                start=(kt == 0), stop=(kt == NKT - 1),
                            )

                    for kt in range(NKT):
                        if qc == 0:
                            if kt % 4 == 0 and kt > 0:
                                qk_proj(kT, wk_t, kt // 4)  # K^T chunk
                            v_proj(kt)  # V tile kt, just before its AV
                        elif kt in (6, 8, 10, 12):
                            # deferred out-projection (late enough that
                            # the previous q-chunk's normalize is done)
                            out_proj(qc - 1, (kt - 6) // 2)
                        sps = s_ps.tile([P, HEADS_PER_CORE, QCHUNK], F32,
                                        tag="sps")
                        # scores: head0 on PE rows 0:64, head1 on 64:128
                        # (concurrent row groups); different PSUM banks
                        for h in range(HEADS_PER_CORE):
                            hsl = slice(h * DH, (h + 1) * DH)
                            nc.tensor.matmul(
                                sps[:, h, :],
                                kT[hsl, kt * P:(kt + 1) * P],
                                qT[hsl, qsl],
                                start=True, stop=True,
                            )
                        # exp on ScalarE, PSUM -> SBUF bf16, both heads
                        pt = ptp.tile([P, HEADS_PER_CORE, QCHUNK],
                                      BF16, tag="pt")
                        nc.scalar.activation(pt, sps, EXP)
                        # AV trails by one k-tile so the PE FIFO never
                        # head-blocks on the next q-chunk's att slot
                        pending.append((kt, pt))
                        if len(pending) > 1:
                            av(*pending.pop(0))
                    av(*pending.pop(0))
                    # stage att to SBUF with one copy so the PSUM slot
                    # frees immediately; normalize from SBUF off-path
                    attsb = small.tile([VW, HEADS_PER_CORE, QCHUNK], F32,
                                       tag="attsb")
                    nc.vector.tensor_copy(attsb, att[0:VW])
                    # both heads' denominator rows -> partition 0, one
                    # approx-reciprocal (reciprocal_approx_fast requires
                    # base_partition 0)
                    den2 = small.tile([1, HEADS_PER_CORE, QCHUNK], F32,
                                      tag="den2")
                    nc.vector.tensor_copy(den2, attsb[DH:DH + 1, :, :])
                    recip2 = small.tile([1, HEADS_PER_CORE, QCHUNK], F32,
                                        tag="recip2")
                    nc.vector.reciprocal_approx_fast(recip2, den2)
                    for h in range(HEADS_PER_CORE):
                        hsl = slice(h * DH, (h + 1) * DH)
                        rb = small.tile([DH, QCHUNK], F32, tag="rb")
                        nc.gpsimd.partition_broadcast(rb, recip2[:, h, :])
                        nc.vector.tensor_tensor(
                            attn[hsl, qsl], attsb[0:DH, h, :], rb, MULT)
                for qt in range(QCHUNK // P):
                    out_proj(NQC - 1, qt)

    nc.compile()
    return nc


def _get_nc(reps=1):
    key = f"nc{reps}"
    if key not in _cached:
        _cached[key] = _build(reps)
    return _cached[key]


def _build_in_maps(inputs):
    x = np.asarray(inputs["x"], dtype=np.float32)
    Wq = np.asarray(inputs["Wq"], dtype=np.float32)
    Wk = np.asarray(inputs["Wk"], dtype=np.float32)
    Wv = np.asarray(inputs["Wv"], dtype=np.float32)
    Wo = np.asarray(inputs["Wo"], dtype=np.float32)

    scale = np.float32(1.0 / np.sqrt(DH))
    in_maps = []
    for core in range(NCORES):
        b = core // CORES_PER_BATCH
        j = core % CORES_PER_BATCH
        csl = slice(j * P, (j + 1) * P)
        bf = ml_dtypes.bfloat16
        in_maps.append({
            "xT": np.ascontiguousarray(x[b].T.astype(bf)),
            "wq": np.ascontiguousarray((Wq[:, csl] * scale).astype(bf)),
            "wk": np.ascontiguousarray(Wk[:, csl].astype(bf)),
            "wv": np.ascontiguousarray(Wv[:, csl].astype(bf)),
            "wo": np.ascontiguousarray(Wo[csl, :].astype(bf)),
        })
    return in_maps


def kernel(x, Wq, Wk, Wv, Wo, bo):
    from concourse import bass_utils

    bo = np.asarray(bo, dtype=np.float32)
    in_maps = _build_in_maps(
        {"x": x, "Wq": Wq, "Wk": Wk, "Wv": Wv, "Wo": Wo})

    res = bass_utils.run_bass_kernel_spmd(
        _get_nc(), in_maps, core_ids=list(range(NCORES)))

    out = np.zeros((B, L, C), dtype=np.float32)
    for core in range(NCORES):
        out[core // CORES_PER_BATCH] += res.results[core]["out"].astype(np.float32)
    out += bo[None, None, :]
    return out
